# revision 7
# baseline (speedup 1.0000x reference)
"""BinaryLinear Trainium2 kernel.

Computes out = x @ sign(weight).T + bias for x [8192, 4096] f32,
weight [4096, 4096] f32, bias [4096] f32.

Strategy: data-parallel over the token dim across 8 NeuronCores
(1024 tokens per core, weight/bias replicated, no collectives).

Per-core pipeline:
  1. x [1024, 4096] f32 -> bf16 (cast during SWDGE DMA, DRAM->DRAM),
     then XBAR DMA-transpose back into SBUF as XT [128i, 32ko, 1024t].
  2. weight rows are cast f32->bf16 DRAM->DRAM in 512-row blocks; each
     [512o, 128i] slab is XBAR-transposed into SBUF [128i, 512o] and
     sign() applied on the Scalar engine (scale=1e30 pushes tiny values
     out of the LUT's zero neighborhood; sign(0)=0 preserved).
  3. TensorE: for each (n, k): 8 matmuls psum[m] += XT[k,m].T @ WT[n,k]
     accumulating fp32 in PSUM over all 32 k-tiles. Bias is added by a
     K=1 fp32 matmul (ones[1,128].T @ bias[1,512]) that initializes each
     PSUM bank, so no broadcast copy is needed.
  4. DVE copies PSUM -> SBUF, HWDGE DMA stores f32 output tiles.
"""

import numpy as np

import concourse.bass as bass
import concourse.mybir as mybir
import concourse.tile as tile
from concourse import bacc
from concourse.bass import ts

P = 128
TOKENS, IN_F, OUT_F = 8192, 4096, 4096
N_CORES = 8
N_TILE = 512  # output-feature tile (one PSUM bank of f32)

F32 = mybir.dt.float32
BF16 = mybir.dt.bfloat16


def build_nc(t_shard=TOKENS // N_CORES, in_f=IN_F, out_f=OUT_F):
    ko_tiles = in_f // P       # k tiles of 128 along in_features
    m_tiles = t_shard // P     # token tiles of 128
    n_tiles = out_f // N_TILE  # output-feature tiles of 512

    nc = bacc.Bacc(None, target_bir_lowering=False, debug=False)

    x = nc.dram_tensor("x", [t_shard, in_f], F32, kind="ExternalInput")
    w = nc.dram_tensor("weight", [out_f, in_f], F32, kind="ExternalInput")
    b = nc.dram_tensor("bias", [out_f], F32, kind="ExternalInput")
    out = nc.dram_tensor("out", [t_shard, out_f], F32, kind="ExternalOutput")

    with tile.TileContext(nc) as tc:
        with (
            tc.tile_pool(name="consts", bufs=1) as const_pool,
            tc.tile_pool(name="xt", bufs=1) as xt_pool,
            tc.tile_pool(name="wt_raw", bufs=10) as wtr_pool,
            tc.tile_pool(name="wt", bufs=10) as wts_pool,
            tc.tile_pool(name="out_sb", bufs=4) as out_pool,
            tc.tile_pool(name="ps", bufs=8, space="PSUM") as psum_pool,
            tc.tile_pool(name="dram", bufs=1, space="DRAM") as dram_pool,
        ):
            # bias replicated across partitions via broadcast DMA
            bias_rep = const_pool.tile([P, out_f], F32)
            nc.sync.dma_start(bias_rep, b[None, :].broadcast_to([P, out_f]))

            # ---- x: cast f32->bf16 (DRAM->DRAM) in column chunks, then
            # transpose each chunk into SBUF right away so the first
            # matmuls are unblocked after one small cast.
            xbf = dram_pool.tile([t_shard, in_f], BF16)
            xt_all = xt_pool.tile([P, ko_tiles, t_shard], BF16)
            kc = 4  # k-tiles per cast chunk
            for c in range(ko_tiles // kc):
                nc.gpsimd.dma_start(
                    xbf[:, ts(c, kc * P)], x[:, ts(c, kc * P)]
                )
                for k in range(c * kc, (c + 1) * kc):
                    nc.sync.dma_start(
                        xt_all[:, k, :], xbf[:, ts(k, P)], transpose=True
                    )

            # ---- weight: cast blocks, transpose+sign, matmul, store
            wbf = dram_pool.tile([out_f, in_f], BF16)
            for n in range(n_tiles):
                nc.gpsimd.dma_start(wbf[ts(n, N_TILE), :], w[ts(n, N_TILE), :])

                psums = [
                    psum_pool.tile([P, N_TILE], F32, name=f"ps_{n}_{m}", tag="ps")
                    for m in range(m_tiles)
                ]

                for k in range(ko_tiles):
                    wt_raw = wtr_pool.tile(
                        [P, N_TILE], BF16, name="wt_raw", tag="wt_raw"
                    )
                    nc.sync.dma_start(
                        wt_raw, wbf[ts(n, N_TILE), ts(k, P)], transpose=True
                    )
                    wt = wts_pool.tile([P, N_TILE], BF16, name="wt", tag="wt")
                    # sign(w); scale pushes tiny magnitudes away from the
                    # LUT's zero breakpoint while keeping sign(0) == 0
                    nc.scalar.activation(
                        wt, wt_raw, mybir.ActivationFunctionType.Sign,
                        scale=1.0e30,
                    )
                    for m in range(m_tiles):
                        nc.tensor.matmul(
                            psums[m],
                            xt_all[:, k, ts(m, P)],
                            wt,
                            start=(k == 0),
                            stop=(k == ko_tiles - 1),
                        )

                for m in range(m_tiles):
                    out_sb = out_pool.tile(
                        [P, N_TILE], F32, name="out_sb", tag="out_sb"
                    )
                    nc.vector.tensor_tensor(
                        out_sb,
                        psums[m],
                        bias_rep[:, ts(n, N_TILE)],
                        mybir.AluOpType.add,
                    )
                    nc.sync.dma_start(out[ts(m, P), ts(n, N_TILE)], out_sb)

    nc.compile()
    return nc


_NC_CACHE = {}


def _get_nc(shape_key):
    if shape_key not in _NC_CACHE:
        _NC_CACHE[shape_key] = build_nc(*shape_key)
    return _NC_CACHE[shape_key]


def kernel(x, weight, bias, _trace=False):
    from concourse.bass_utils import run_bass_kernel_spmd

    x = np.ascontiguousarray(np.asarray(x, dtype=np.float32))
    weight = np.ascontiguousarray(np.asarray(weight, dtype=np.float32))
    bias = np.ascontiguousarray(np.asarray(bias, dtype=np.float32))

    tokens = x.shape[0]
    t_shard = tokens // N_CORES
    nc = _get_nc((t_shard, x.shape[1], weight.shape[0]))

    in_maps = [
        {
            "x": x[c * t_shard : (c + 1) * t_shard],
            "weight": weight,
            "bias": bias,
        }
        for c in range(N_CORES)
    ]
    res = run_bass_kernel_spmd(
        nc, in_maps, core_ids=list(range(N_CORES)), trace=_trace
    )
    out = np.concatenate([r["out"] for r in res.results], axis=0)
    if _trace:
        return out, res
    return out


# revision 25
# speedup vs baseline: 1.4159x; 1.4159x over previous
"""BinaryLinear Trainium2 kernel.

Computes out = x @ sign(weight).T + bias for x [8192, 4096] f32,
weight [4096, 4096] f32, bias [4096] f32.

Strategy: data-parallel over the token dim across 8 NeuronCores
(1024 tokens per core, weight/bias replicated, no collectives).

Per-core pipeline (no DRAM scratch):
  1. x tiles [128t, 4096i] are cast f32->bf16 during the SWDGE DMA load,
     then one whole-tile XBAR transpose SBUF->SBUF lands each in
     XT [128i, 32k, 1024t] (8 transposes total for x).
  2. weight rows likewise: cast to bf16 SBUF slabs [128o, 4096i]; one
     XBAR transpose per slab fills WT_n [128i, 32k, 512o] (4 per output
     block); sign() is applied in place on the Scalar engine (scale=1e30
     pushes tiny values off the LUT's zero neighborhood; sign(0)=0
     preserved). Few, large transposes minimize XBAR-mode serialization
     against other DMA traffic.
  3. TensorE: psum[m] += XT[k,m].T @ WT[n,k], fp32 accumulation in PSUM
     over all 32 k-tiles; 8 token tiles <-> 8 PSUM banks.
  4. DVE adds the (partition-broadcast) bias while copying PSUM->SBUF;
     HWDGE stores f32 output tiles.
"""

import numpy as np

import concourse.mybir as mybir
import concourse.tile as tile
from concourse import bacc
from concourse.bass import ts

P = 128
TOKENS, IN_F, OUT_F = 8192, 4096, 4096
N_CORES = 8
N_TILE = 512   # output-feature block (one PSUM bank of f32)

F32 = mybir.dt.float32
BF16 = mybir.dt.bfloat16


def build_nc(t_shard=TOKENS // N_CORES, in_f=IN_F, out_f=OUT_F, repeat=1):
    m_tiles = t_shard // P      # token tiles of 128
    n_tiles = out_f // N_TILE   # output blocks of 512
    ko_tiles = in_f // P        # k tiles of 128
    j_tiles = N_TILE // P       # 128-row slabs per output block

    nc = bacc.Bacc(None, target_bir_lowering=False, debug=False)

    x = nc.dram_tensor("x", [t_shard, in_f], F32, kind="ExternalInput")
    w = nc.dram_tensor("weight", [out_f, in_f], F32, kind="ExternalInput")
    b = nc.dram_tensor("bias", [out_f], F32, kind="ExternalInput")
    out = nc.dram_tensor("out", [t_shard, out_f], F32, kind="ExternalOutput")

    with tile.TileContext(nc) as tc:
        with (
            tc.tile_pool(name="consts", bufs=2) as const_pool,
            tc.tile_pool(name="stage", bufs=6) as stage_pool,
            tc.tile_pool(name="xt", bufs=1) as xt_pool,
            tc.tile_pool(name="wt", bufs=2) as wt_pool,
            tc.tile_pool(name="out_sb", bufs=3) as out_pool,
            tc.tile_pool(name="ps", bufs=8, space="PSUM") as psum_pool,
        ):
          for _rep in range(repeat):

            def cast_slab(src_rows):
                """SWDGE cast f32->bf16 of 128 DRAM rows into SBUF."""
                slab = stage_pool.tile([P, in_f], BF16, name="slab", tag="stage")
                nc.gpsimd.dma_start(slab, src_rows)
                return slab

            def emit_wt(n):
                """Build signed WT tile [128i, 32k, 512o] for block n."""
                wt_n = wt_pool.tile(
                    [P, ko_tiles, N_TILE], BF16, name=f"wt_{n}", tag="wt"
                )
                slabs = [
                    cast_slab(w[ts(n * j_tiles + j, P), :])
                    for j in range(j_tiles)
                ]
                for j in range(j_tiles):
                    nc.sync.dma_start(
                        wt_n[:, :, ts(j, P)], slabs[j], transpose=True
                    )
                # sign in place; scale pushes tiny magnitudes off the LUT's
                # zero breakpoint while keeping sign(0) == 0
                nc.scalar.activation(
                    wt_n, wt_n, mybir.ActivationFunctionType.Sign, scale=1.0e30
                )
                return wt_n

            # ---- head: W block 0 first (it gates the first matmuls),
            # then x tiles (each transposed whole so matmuls can start
            # after the first).
            wts = {0: emit_wt(0)}
            xt_all = xt_pool.tile(
                [P, ko_tiles, t_shard], BF16, name="xt_all", tag="xt"
            )
            for m in range(m_tiles):
                slab = cast_slab(x[ts(m, P), :])
                nc.sync.dma_start(
                    xt_all[:, :, ts(m, P)], slab, transpose=True
                )
                if m == 0 and n_tiles > 1:
                    wts[1] = emit_wt(1)

            # ---- main loop over output blocks
            for n in range(n_tiles):
                if n + 2 < n_tiles:
                    wts[n + 2] = emit_wt(n + 2)
                wt_n = wts.pop(n)

                # per-block bias, replicated across partitions via DMA
                bias_rep = const_pool.tile(
                    [P, N_TILE], F32, name="bias_rep", tag="bias"
                )
                nc.sync.dma_start(
                    bias_rep,
                    b[None, ts(n, N_TILE)].broadcast_to([P, N_TILE]),
                )

                # two phase-shifted groups of 4 PSUM banks: group B's
                # matmuls overlap group A's output copies
                half = max(1, m_tiles // 2)
                for g0 in range(0, m_tiles, half):
                    ms = range(g0, min(g0 + half, m_tiles))
                    psums = {
                        m: psum_pool.tile(
                            [P, N_TILE], F32, name=f"ps_{n}_{m}", tag="ps"
                        )
                        for m in ms
                    }
                    for k in range(ko_tiles):
                        for m in ms:
                            nc.tensor.matmul(
                                psums[m],
                                xt_all[:, k, ts(m, P)],
                                wt_n[:, k, :],
                                start=(k == 0),
                                stop=(k == ko_tiles - 1),
                            )
                    for m in ms:
                        out_sb = out_pool.tile(
                            [P, N_TILE], F32, name="out_sb", tag="out_sb"
                        )
                        nc.vector.tensor_tensor(
                            out_sb, psums[m], bias_rep, mybir.AluOpType.add
                        )
                        nc.sync.dma_start(
                            out[ts(m, P), ts(n, N_TILE)], out_sb
                        )

    nc.compile()
    return nc


_NC_CACHE = {}


def _get_nc(shape_key):
    if shape_key not in _NC_CACHE:
        _NC_CACHE[shape_key] = build_nc(*shape_key)
    return _NC_CACHE[shape_key]


def kernel(x, weight, bias, _trace=False):
    from concourse.bass_utils import run_bass_kernel_spmd

    x = np.ascontiguousarray(np.asarray(x, dtype=np.float32))
    weight = np.ascontiguousarray(np.asarray(weight, dtype=np.float32))
    bias = np.ascontiguousarray(np.asarray(bias, dtype=np.float32))

    tokens = x.shape[0]
    t_shard = tokens // N_CORES
    nc = _get_nc((t_shard, x.shape[1], weight.shape[0]))

    in_maps = [
        {
            "x": x[c * t_shard : (c + 1) * t_shard],
            "weight": weight,
            "bias": bias,
        }
        for c in range(N_CORES)
    ]
    res = run_bass_kernel_spmd(
        nc, in_maps, core_ids=list(range(N_CORES)), trace=_trace
    )
    out = np.concatenate([r["out"] for r in res.results], axis=0)
    if _trace:
        return out, res
    return out
